# revision 1
# baseline (speedup 1.0000x reference)
"""Trainium2 Bass kernel for nn_Block_17738214932786 (spiking transformer block).

Computation (B=16, C=512, N=1024, H=8 heads, HID=2048):
    q = spike(bn(q_w @ x)); k,v likewise          (spikes are 0/1)
    attn = (Qh Kh^T) Vh * 0.25 == Qh (Kh^T Vh) * 0.25   (exact: integers)
    a = spike(attn)                               (threshold attn >= 8)
    a = spike(bn(proj_w @ a + proj_bias))
    x = x + a
    h = spike(bn(fc1_w @ x + fc1_bias))
    h = spike(bn(fc2_w @ h + fc2_bias))
    out = x + h

Strategy: data-parallel over batch across 8 NeuronCores (2 batches/core,
per-core activation matrix [512, 2048]). BatchNorm (training mode: stats
over batch*length) is handled sync-BN style: per-channel [mean, E[y^2]]
AllGathered per conv (tiny buffers); BN+LIF then collapses to a
per-channel threshold compare y >= t.

Precision: float32r (TF32-like, 11-bit mantissa) runs at 1 PE cycle/row
for >=256 moving cols — same rate as bf16, vs 2-4x for fp32. qkv convs:
3-pass f32r hi/lo splits of weights AND x (error ~2^-23, splits prepped
on host). Attention: exact (spikes 0/1 in fp8/bf16; KtV integer counts
<=1024 split hi/lo bf16 losslessly; PSUM fp32). proj: 2-pass bf16 hi/lo
weights (~2^-17; rhs binary exact). fc1: single-pass f32r on f32r(x+a)
(~2^-12 both sides). fc2: single-pass f32r weights (~2^-12; rhs h
binary, exact in f32r). Simulated end-to-end rel err 9.4e-3 (gate 2e-2).
"""

import sys
import types
import numpy as np

B, C, N, H = 16, 512, 1024, 8
D = C // H
HID = 4 * C
NCORES = 8
BPC = B // NCORES          # batches per core
COLS = BPC * N             # 2048
P = 128
NKC = C // P               # 4  tiles over C
NMH = HID // P             # 16 tiles over HID
NCH = COLS // 512          # 4  512-col chunks per core
NPT = COLS // P            # 16 col tiles per core
BN_EPS = 1e-5

_cache = {}


def _ensure_axon_hooks_shim():
    try:
        import antenv.axon_hooks  # noqa: F401
        return
    except Exception:
        pass
    m = types.ModuleType("antenv.axon_hooks")
    m.get_axon_ntff_profile_hook = lambda: None
    try:
        import antenv  # noqa: F401
    except Exception:
        sys.modules["antenv"] = types.ModuleType("antenv")
    sys.modules["antenv.axon_hooks"] = m


def _build_program():
    from contextlib import ExitStack
    import concourse.bacc as bacc
    import concourse.tile as tile
    from concourse import mybir
    from concourse.masks import make_identity

    dt = mybir.dt
    f32, bf16, f32r = dt.float32, dt.bfloat16, dt.float32r
    f8 = dt.float8e4
    AF = mybir.ActivationFunctionType
    GE = mybir.AluOpType.is_ge
    RG = [list(range(NCORES))]

    nc = bacc.Bacc("TRN2", target_bir_lowering=False, debug=False,
                   num_devices=NCORES)

    xh_in = nc.dram_tensor("x_hi", [C, COLS], f32r, kind="ExternalInput")
    xl_in = nc.dram_tensor("x_lo", [C, COLS], f32r, kind="ExternalInput")
    xf_in = nc.dram_tensor("x_f32", [C, COLS], f32, kind="ExternalInput")
    wqkvh_in = nc.dram_tensor("wqkvT_hi", [C, 3 * C], f32r, kind="ExternalInput")
    wqkvl_in = nc.dram_tensor("wqkvT_lo", [C, 3 * C], f32r, kind="ExternalInput")
    wph_in = nc.dram_tensor("wprojT_hi", [C, C], bf16, kind="ExternalInput")
    wpl_in = nc.dram_tensor("wprojT_lo", [C, C], bf16, kind="ExternalInput")
    wfc1_in = nc.dram_tensor("wfc1T", [C, HID], f32r, kind="ExternalInput")
    wfc2_in = nc.dram_tensor("wfc2T", [HID, C], f32r, kind="ExternalInput")
    thr_qkv_in = nc.dram_tensor("thr_qkv", [C, 6], f32, kind="ExternalInput")
    thr_proj_in = nc.dram_tensor("thr_proj", [C, 2], f32, kind="ExternalInput")
    thr_fc1_in = nc.dram_tensor("thr_fc1", [HID, 2], f32, kind="ExternalInput")
    thr_fc2_in = nc.dram_tensor("thr_fc2", [C, 2], f32, kind="ExternalInput")
    out_ext = nc.dram_tensor("out", [C, COLS], f32, kind="ExternalOutput")

    def part3(ap, p=P):  # [(m p), n] dram view -> [p, m, n]
        return ap.rearrange("(m p) n -> p m n", p=p)

    with tile.TileContext(nc, pool_alloc_mode="queue") as tc, ExitStack() as es:
        misc = es.enter_context(tc.tile_pool(name="misc", bufs=1))
        dram = es.enter_context(tc.tile_pool(name="dram", bufs=1, space="DRAM"))
        pp_mm = es.enter_context(tc.tile_pool(name="pp_mm", bufs=6, space="PSUM"))
        pp_sm = es.enter_context(tc.tile_pool(name="pp_sm", bufs=2, space="PSUM"))

        ident_bf = misc.tile([P, P], bf16)
        make_identity(nc, ident_bf)
        eps_t = misc.tile([P, 1], f32)
        nc.vector.memset(eps_t, BN_EPS)

        par_qkv = misc.tile([P, NKC, 6], f32)
        nc.gpsimd.dma_start(out=par_qkv, in_=part3(thr_qkv_in[:, :]))
        par_proj = misc.tile([P, NKC, 2], f32)
        nc.gpsimd.dma_start(out=par_proj, in_=part3(thr_proj_in[:, :]))
        par_fc1 = misc.tile([P, NMH, 2], f32)
        nc.gpsimd.dma_start(out=par_fc1, in_=part3(thr_fc1_in[:, :]))
        par_fc2 = misc.tile([P, NKC, 2], f32)
        nc.gpsimd.dma_start(out=par_fc2, in_=part3(thr_fc2_in[:, :]))

        # warmup collective: absorbs the first-call staging/skew cost so the
        # k-conv stats AllGather runs at steady-state latency
        dmy_in = dram.tile([P, 2], f32, name="dmy_in")
        dmy_out = dram.tile([NCORES, P, 2], f32, name="dmy_out")
        nc.gpsimd.collective_compute(
            "AllGather", mybir.AluOpType.bypass, replica_groups=RG,
            ins=[dmy_in.opt()], outs=[dmy_out.opt()])

        xres_d = dram.tile([NKC, P, COLS], f32, name="xres_dram")
        h1_d = dram.tile([NMH, P, COLS], f32r, name="h1_dram")

        def stats_finish(name, pool, stats, nm):
            mv = pool.tile([P, nm, 2], f32, name=f"mv_{name}")
            for m in range(nm):
                nc.vector.bn_aggr(out=mv[:, m, :], in_=stats[:, m, :, :])
            pack = pool.tile([P, nm, 2], f32, name=f"pk_{name}")
            nc.vector.tensor_mul(pack[:, :, 1], mv[:, :, 0], mv[:, :, 0])
            nc.vector.tensor_add(pack[:, :, 1], pack[:, :, 1], mv[:, :, 1])
            nc.vector.tensor_copy(pack[:, :, 0], mv[:, :, 0])
            bin_ = dram.tile([P, nm * 2], f32, name=f"arin_{name}")
            bout = dram.tile([NCORES, P, nm * 2], f32, name=f"arout_{name}")
            nc.sync.dma_start(out=bin_, in_=pack)
            nc.gpsimd.collective_compute(
                "AllGather", mybir.AluOpType.bypass, replica_groups=RG,
                ins=[bin_.opt()], outs=[bout.opt()])
            return bout

        def thresholds(name, pool, bout, thr_par, thr_col, nm):
            """AllGathered per-core stats -> reduce -> thresholds [P, nm]."""
            ag = pool.tile([P, NCORES, nm, 2], f32, name=f"ag_{name}")
            nc.sync.dma_start(out=ag, in_=bout.rearrange("r p c -> p r c"))
            # tree-reduce over the 8 ranks
            nc.vector.tensor_add(ag[:, 0:4], ag[:, 0:4], ag[:, 4:8])
            nc.vector.tensor_add(ag[:, 0:2], ag[:, 0:2], ag[:, 2:4])
            arst = pool.tile([P, nm, 2], f32, name=f"ar_{name}")
            nc.vector.tensor_add(arst, ag[:, 0, :, :], ag[:, 1, :, :])
            nc.vector.tensor_scalar_mul(arst, arst, 1.0 / NCORES)
            t_t = pool.tile([P, nm], f32, name=f"thr_{name}")
            tmp = pool.tile([P, nm], f32, name=f"tmp_{name}")
            nc.vector.tensor_mul(tmp, arst[:, :, 0], arst[:, :, 0])
            nc.vector.tensor_sub(tmp, arst[:, :, 1], tmp)
            nc.scalar.activation(out=tmp, in_=tmp, func=AF.Sqrt,
                                 bias=eps_t, scale=1.0)
            nc.vector.tensor_mul(tmp, tmp, thr_par[:, :, thr_col])
            nc.vector.tensor_add(t_t, tmp, arst[:, :, 0])
            nc.vector.tensor_sub(t_t, t_t, thr_par[:, :, thr_col + 1])
            return t_t

        def spike(dst, src, t_t, tcol):
            nc.vector.tensor_scalar(out=dst, in0=src,
                                    scalar1=t_t[:, tcol:tcol + 1],
                                    scalar2=None, op0=GE)

        def transposes(spkb, dstT):
            # bf16 PE transpose; the PSUM->SBUF copy casts to fp8 (0/1 exact)
            for m in range(NKC):
                for p_ in range(NPT):
                    pst = pp_sm.tile([P, P], bf16, name="ps_sm")
                    nc.tensor.transpose(pst, in_=spkb[:, m, P * p_:P * p_ + P],
                                        identity=ident_bf)
                    nc.any.tensor_copy(dstT[:, p_, P * m:P * m + P], pst)

        def phase_ab(a_spk, p_ab):
            q_spk = p_ab.tile([P, NKC, COLS], bf16)
            kT = p_ab.tile([P, NPT, C], f8)
            vT = p_ab.tile([P, NPT, C], f8)

            # ---- Phase A: k,v,q convs (3-pass f32r), interleaved so
            # every AllGather hides under the next conv's matmuls ----
            with tc.tile_pool(name="p_ykv", bufs=2) as p_ykv:
                with tc.tile_pool(name="p_w", bufs=2) as p_w, \
                     tc.tile_pool(name="p_x", bufs=3) as p_x, \
                     tc.tile_pool(name="p_spk", bufs=1) as p_spk:
                    def qkv_conv(ci, y_sb):
                        wh = p_w.tile([P, NKC, C], f32r, name="w_qkvh",
                                      bufs=2)
                        for hf2 in range(2):
                            c2 = 512 * ci + 256 * hf2
                            (nc.scalar, nc.gpsimd)[hf2].dma_start(
                                out=wh[:, :, 256 * hf2:256 * hf2 + 256],
                                in_=part3(wqkvh_in[:, :])[:, :, c2:c2 + 256])
                        wl = p_w.tile([P, NKC, C], f32r, name="w_qkvl",
                                      bufs=2)
                        nc.scalar.dma_start(
                            out=wl,
                            in_=part3(wqkvl_in[:, :])[:, :, 512 * ci:512 * ci + 512])
                        st = misc.tile([P, NKC, NCH, 6], f32, name=f"st_qkv{ci}")
                        for hf in range(NCH):
                            cs = slice(512 * hf, 512 * hf + 512)
                            # x_hi and x_lo chunks never coexist: passes 1-2
                            # for all m run on x_hi across 4 open PSUM banks,
                            # then pass 3 adds the x_lo term.
                            xh = p_x.tile([P, NKC, 512], f32r, name="xc", bufs=3)
                            nc.sync.dma_start(out=xh,
                                              in_=part3(xh_in[:, :])[:, :, cs])
                            pss = [pp_mm.tile([P, 512], f32, name="ps_mm")
                                   for _ in range(NKC)]
                            for m in range(NKC):
                                for pi, wt in enumerate((wh, wl)):
                                    for k in range(NKC):
                                        nc.tensor.matmul(
                                            pss[m],
                                            lhsT=wt[:, k, P * m:P * m + P],
                                            rhs=xh[:, k, :],
                                            start=(pi == 0 and k == 0),
                                            stop=False)
                            xl = p_x.tile([P, NKC, 512], f32r, name="xc", bufs=3)
                            nc.scalar.dma_start(out=xl,
                                                in_=part3(xl_in[:, :])[:, :, cs])
                            for m in range(NKC):
                                for k in range(NKC):
                                    nc.tensor.matmul(
                                        pss[m],
                                        lhsT=wh[:, k, P * m:P * m + P],
                                        rhs=xl[:, k, :],
                                        start=False,
                                        stop=(k == NKC - 1))
                                nc.any.tensor_copy(y_sb[:, m, cs], pss[m])
                                nc.vector.bn_stats(out=st[:, m, hf, :], in_=pss[m])
                        return stats_finish(f"qkv{ci}", misc, st, NKC)

                    y_k = p_ykv.tile([P, NKC, COLS], f32, name="ybuf")
                    bout_k = qkv_conv(1, y_k)
                    y_v = p_ykv.tile([P, NKC, COLS], f32, name="ybuf")
                    bout_v = qkv_conv(2, y_v)
                    # k spikes (fp8) + transposes (AR-k already done)
                    t_k = thresholds("k", misc, bout_k, par_qkv, 2, NKC)
                    spkb_k = p_spk.tile([P, NKC, COLS], bf16, name="spkb",
                                        bufs=1)
                    for m in range(NKC):
                        spike(spkb_k[:, m, :], y_k[:, m, :], t_k, m)
                    transposes(spkb_k, kT)
                    y_q = p_ykv.tile([P, NKC, COLS], f32, name="ybuf")
                    bout_q = qkv_conv(0, y_q)
                    # v spikes + transposes (AR-v hidden under q conv)
                    t_v = thresholds("v", misc, bout_v, par_qkv, 4, NKC)
                    spkb_v = p_spk.tile([P, NKC, COLS], bf16, name="spkb",
                                        bufs=1)
                    for m in range(NKC):
                        spike(spkb_v[:, m, :], y_v[:, m, :], t_v, m)
                    transposes(spkb_v, vT)
                    # q spikes (DVE waits AR-q; PE continues with
                    # v-transposes and the kv matmuls of phase B)
                    t_q = thresholds("q", misc, bout_q, par_qkv, 0, NKC)
                    for m in range(NKC):
                        spike(q_spk[:, m, :], y_q[:, m, :], t_q, m)

            # ---- Phase B: attention (exact integer bf16/fp8) ----
            with tc.tile_pool(name="p_kv", bufs=4) as p_kv:
                kvs = {}
                for b in range(BPC):
                    for j in range(H // 2):   # head pairs -> blockdiag lhsT
                        blk_hi = p_kv.tile([P, P], bf16, name="kvblk_hi")
                        blk_lo = p_kv.tile([P, P], bf16, name="kvblk_lo")
                        nc.vector.memset(blk_hi, 0.0)
                        nc.vector.memset(blk_lo, 0.0)
                        pkv = pp_sm.tile([P, 64], f32, name="ps_sm")
                        for hh in range(2):
                            h_ = 2 * j + hh
                            sl = slice(64 * hh, 64 * hh + 64)
                            for t_ in range(N // P):
                                nc.tensor.matmul(
                                    pkv[sl, :],
                                    lhsT=kT[:, (N // P) * b + t_, D * h_:D * h_ + D],
                                    rhs=vT[:, (N // P) * b + t_, D * h_:D * h_ + D],
                                    start=(t_ == 0), stop=(t_ == N // P - 1),
                                    tile_position=(0, 64 * hh))
                            # lossless integer split: hi=bf16(kv), lo=kv-hi
                            nc.any.tensor_copy(blk_hi[sl, sl], pkv[sl, :])
                            nc.vector.tensor_sub(blk_lo[sl, sl], pkv[sl, :],
                                                 blk_hi[sl, sl])
                        kvs[(b, j)] = (blk_hi, blk_lo)

                for b in range(BPC):
                    for j in range(H // 2):
                        blk_hi, blk_lo = kvs[(b, j)]
                        pas = [pp_mm.tile([P, 512], f32, name="ps_mm")
                               for _ in range(N // 512)]
                        for wi, blk in enumerate((blk_hi, blk_lo)):
                            for n_ in range(N // 512):
                                cs = slice(N * b + 512 * n_, N * b + 512 * n_ + 512)
                                nc.tensor.matmul(pas[n_], lhsT=blk,
                                                 rhs=q_spk[:, j, cs],
                                                 start=(wi == 0), stop=(wi == 1))
                        for n_ in range(N // 512):
                            cs = slice(N * b + 512 * n_, N * b + 512 * n_ + 512)
                            nc.vector.tensor_scalar(
                                out=a_spk[:, j, cs], in0=pas[n_],
                                scalar1=8.0, scalar2=None, op0=GE)

        def phase_c(a_spk, xrr, w1q0):
            # ---- Phase C: proj (2-pass bf16) + residual -> xres_d; also
            # build xrr (f32r of x+a2) in SBUF for fc1 ----
            with tc.tile_pool(name="p_pr", bufs=1) as p_pr:
                wpT_hi = p_pr.tile([P, NKC, C], bf16)
                nc.sync.dma_start(out=wpT_hi, in_=part3(wph_in[:, :]))
                wpT_lo = p_pr.tile([P, NKC, C], bf16)
                nc.sync.dma_start(out=wpT_lo, in_=part3(wpl_in[:, :]))
                # prefetch the residual x tiles and the full fc1 weights
                # (no deps - hide under proj matmuls / the stats AllGather)
                xcs = []
                for m in range(NKC):
                    xc = p_pr.tile([P, COLS], f32, name=f"xc_res{m}")
                    nc.scalar.dma_start(out=xc, in_=part3(xf_in[:, :])[:, m, :])
                    xcs.append(xc)
                # prefetch fc1 slice-0 weights (would otherwise sit on the
                # proj-AR bridge critical path)
                for half in range(2):
                    eng = (nc.sync, nc.gpsimd)[half]
                    eng.dma_start(
                        out=w1q0[:, :, 512 * half:512 * half + 512],
                        in_=part3(wfc1_in[:, :])[:, :, 512 * half:512 * half + 512])


                y_p = p_pr.tile([P, NKC, COLS], f32)
                st_p = misc.tile([P, NKC, NCH, 6], f32, name="st_proj")
                for hf in range(2):
                    for mi in range(2):
                        m = 2 * hf + mi
                        pss = [pp_mm.tile([P, 512], f32, name="ps_mm")
                               for _ in range(NCH)]
                        for wi, wt in enumerate((wpT_hi, wpT_lo)):
                            for k in range(NKC):
                                for n_ in range(NCH):
                                    nc.tensor.matmul(
                                        pss[n_], lhsT=wt[:, k, P * m:P * m + P],
                                        rhs=a_spk[:, k, 512 * n_:512 * n_ + 512],
                                        start=(wi == 0 and k == 0),
                                        stop=(wi == 1 and k == NKC - 1))
                        for n_ in range(NCH):
                            nc.any.tensor_copy(y_p[:, m, 512 * n_:512 * n_ + 512],
                                               pss[n_])
                            nc.vector.bn_stats(out=st_p[:, m, n_, :], in_=pss[n_])
                # single AllGather for all 4 m-tiles: two serialized CC
                # ops cost more than one slightly-later one
                bout_p = stats_finish("proj", misc, st_p, NKC)
                # PE<->DVE ping-pong keep-alive: each matmul depends on a
                # copy of the previous PSUM, pacing ~1us/round so the PE
                # clock stays ramped through the AllGather wait
                wka = p_pr.tile([P, P], bf16, name="wka")
                nc.vector.tensor_copy(wka, ident_bf)
                wpsk = pp_sm.tile([P, P], f32, name="ps_sm")
                for _ in range(10):
                    nc.tensor.matmul(wpsk, lhsT=wka, rhs=wka,
                                     start=True, stop=True)
                    nc.vector.tensor_copy(wka, wpsk)
                t_p = thresholds("proj", misc, bout_p, par_proj, 0, NKC)
                f32_ = mybir.dt.float32
                for m in range(NKC):
                    spike(y_p[:, m, :], y_p[:, m, :], t_p, m)
                    # fused add + f32r round; the final residual then uses
                    # f32r(x + a), a 2^-12 error on the x term (negligible)
                    nc.vector.tensor_add(xrr[:, m, :], y_p[:, m, :], xcs[m])
                    eng = (nc.sync, nc.scalar)[m % 2]
                    eng.dma_start(out=xres_d[m, :, :],
                                  in_=xrr[:, m, :].bitcast(f32_))
            return t_p

        def phase_d(xrr, t_ps, p_xrr, w1q0):
            # ====== Phase D: fc1 (1-pass f32r) in 2 slices of 8 ======
            # Each slice's ~34us of matmuls fully hides one stats
            # AllGather; slice 0's spike work runs during slice 1.
            SLICES = ((0, 8), (8, 8))
            with tc.tile_pool(name="p_f1q", bufs=2) as p_f1q:
                # PE warm-keeper: dummy matmuls gated on the proj
                # thresholds run during the bridge so fc1 starts at
                # full clock instead of HAM-cold.
                warm = misc.tile([P, 512], bf16, name="warm")
                nc.vector.tensor_copy(warm[:, 0:2], t_ps[:, 0:2])
                wps = pp_mm.tile([P, 512], f32, name="ps_mm")
                for _ in range(24):
                    nc.tensor.matmul(wps[:, 0:P], lhsT=warm[:, 0:P],
                                     rhs=warm[:, 0:P],
                                     start=True, stop=True)

                def finish_slice(qt, m0, qm, bout_q, y1q):
                    t1q = thresholds(f"fc1q{qt}", misc, bout_q,
                                     par_fc1[:, m0:m0 + qm, :], 0, qm)
                    for mi in range(qm):
                        h1v = y1q[:, mi, :].bitcast(f32r)
                        spike(h1v, y1q[:, mi, :], t1q, mi)
                        eng = (nc.sync, nc.scalar, nc.gpsimd)[mi % 3]
                        eng.dma_start(out=h1_d[m0 + mi, :, :], in_=h1v)

                pend = None
                for qt, (m0, qm) in enumerate(SLICES):
                    if qt == 0:
                        w1q = w1q0
                    else:
                        w1q = p_xrr.tile([P, NKC, 1024], f32r, name="w1q",
                                         bufs=1)
                        for half in range(2):
                            eng = (nc.sync, nc.scalar)[half]
                            c0 = 1024 + 512 * half
                            eng.dma_start(
                                out=w1q[:, :, 512 * half:512 * half + 512],
                                in_=part3(wfc1_in[:, :])[:, :, c0:c0 + 512])
                    y1q = p_f1q.tile([P, qm, COLS], f32, name="y1q")
                    st_q = misc.tile([P, qm, NCH, 6], f32,
                                     name=f"st_fc1q{qt}")
                    for mi in range(qm):
                        pss = [pp_mm.tile([P, 512], f32, name="ps_mm")
                               for _ in range(NCH)]
                        for k in range(NKC):
                            for n_ in range(NCH):
                                nc.tensor.matmul(
                                    pss[n_],
                                    lhsT=w1q[:, k, P * mi:P * mi + P],
                                    rhs=xrr[:, k, 512 * n_:512 * n_ + 512],
                                    start=(k == 0), stop=(k == NKC - 1))
                        for n_ in range(NCH):
                            nc.any.tensor_copy(
                                y1q[:, mi, 512 * n_:512 * n_ + 512], pss[n_])
                            nc.vector.bn_stats(out=st_q[:, mi, n_, :],
                                               in_=pss[n_])
                    bout_q = stats_finish(f"fc1q{qt}", misc, st_q, qm)
                    if pend is not None:
                        finish_slice(*pend)
                    pend = (qt, m0, qm, bout_q, y1q)
                wkb = misc.tile([P, P], bf16, name="wkb")
                nc.vector.tensor_copy(wkb, warm[:, 0:P])
                wpsk2 = pp_sm.tile([P, P], f32, name="ps_sm")
                for _ in range(9):
                    nc.tensor.matmul(wpsk2, lhsT=wkb, rhs=wkb,
                                     start=True, stop=True)
                    nc.vector.tensor_copy(wkb, wpsk2)
                finish_slice(*pend)

        def phase_e():
            # ====== Phase E: fc2 (1-pass f32r, chunk-outer, one AR) ======
            # Initial loads spread across the 3 DMA-capable queues; h1
            # streams per fc1-slice so chunk 0's first 12 k-tiles can run
            # before fc1's last slice lands in DRAM.
            with tc.tile_pool(name="p_f2", bufs=1) as p_f2, \
                 tc.tile_pool(name="p_f2h", bufs=3) as p_f2h:
                engs = (nc.sync, nc.scalar)
                wfc2T = p_f2.tile([P, NMH, C], f32r)
                for sl_ in range(4):
                    nc.gpsimd.dma_start(
                        out=wfc2T[:, 4 * sl_:4 * sl_ + 4, :],
                        in_=part3(wfc2_in[:, :])[:, 4 * sl_:4 * sl_ + 4, :])

                y2 = p_f2.tile([P, NKC, COLS], f32)
                st2 = misc.tile([P, NKC, NCH, 6], f32, name="st_fc2")
                out3 = part3(out_ext[:, :])
                h1r = h1_d.rearrange("m p n -> p m n")
                SLICES = ((0, 8), (8, 4), (12, 4))
                xrcs = []
                warm2 = p_f2.tile([P, P], f32r, name="warm2")
                wps2 = pp_sm.tile([P, P], f32, name="ps_sm")
                for n_ in range(NCH):
                    cs = slice(512 * n_, 512 * n_ + 512)
                    h1cs = []
                    for sl_, (m0, qm) in enumerate(SLICES):
                        h1c = p_f2h.tile([P, qm, 512], f32r, name=f"h1c{sl_}")
                        engs[(sl_ + n_) % 2].dma_start(
                            out=h1c, in_=h1r[:, m0:m0 + qm, cs])
                        h1cs.append(h1c)
                        if n_ == 0:
                            # clock warm-keeper: runs as each piece lands
                            nc.vector.tensor_copy(warm2, h1c[:, 0, 0:P])
                            for _ in range(8):
                                nc.tensor.matmul(wps2, lhsT=warm2,
                                                 rhs=warm2,
                                                 start=True, stop=True)
                    for m in range(NKC):
                        ps = pp_mm.tile([P, 512], f32, name="ps_mm")
                        for sl_, (m0, qm) in enumerate(SLICES):
                            for k in range(qm):
                                nc.tensor.matmul(
                                    ps,
                                    lhsT=wfc2T[:, m0 + k, P * m:P * m + P],
                                    rhs=h1cs[sl_][:, k, :],
                                    start=(sl_ == 0 and k == 0),
                                    stop=(sl_ == 2 and k == qm - 1))
                        nc.any.tensor_copy(y2[:, m, cs], ps)
                        nc.vector.bn_stats(out=st2[:, m, n_, :], in_=ps)
                    if n_ == 0:
                        # residual tiles: no deps on this phase's compute,
                        # loads hide under the remaining chunks
                        for m in range(NKC):
                            xrc = p_f2.tile([P, COLS], f32, name=f"xrc{m}")
                            engs[m % 2].dma_start(out=xrc, in_=xres_d[m, :, :])
                            xrcs.append(xrc)
                bout2 = stats_finish("fc2", misc, st2, NKC)
                t2 = thresholds("fc2", misc, bout2, par_fc2, 0, NKC)
                # chunked spike+add+store so the first output DMAs launch
                # while later chunks are still in the vector queue
                for n_ in range(NCH):
                    cs = slice(512 * n_, 512 * n_ + 512)
                    for m in range(NKC):
                        spike(y2[:, m, cs], y2[:, m, cs], t2, m)
                        nc.vector.tensor_add(y2[:, m, cs], y2[:, m, cs],
                                             xrcs[m][:, cs])
                        engs[(m + n_) % 2].dma_start(out=out3[:, m, cs],
                                                     in_=y2[:, m, cs])

        with tc.tile_pool(name="p_as", bufs=1) as p_as:  # a_spk: A..C
            a_spk = p_as.tile([P, NKC, COLS], bf16)
            with tc.tile_pool(name="p_ab", bufs=1) as p_ab:  # lives A..B
                phase_ab(a_spk, p_ab)
            with tc.tile_pool(name="p_xrr", bufs=1) as p_xrr:  # lives C..D
                xrr = p_xrr.tile([P, NKC, COLS], f32r)
                w1q0 = p_xrr.tile([P, NKC, 1024], f32r, name="w1q", bufs=1)
                t_ps = phase_c(a_spk, xrr, w1q0)
                phase_d(xrr, t_ps, p_xrr, w1q0)
        phase_e()

    nc.compile()
    return nc


def _f32r(v):
    """Round float32 array to f32r (11-bit mantissa, RNE) — bit-exact vs
    the TRN2 DVE cast (verified on hardware)."""
    x = np.ascontiguousarray(v, np.float32).view(np.uint32)
    keep = np.uint32(0xFFFFF000)
    half = np.uint32(0x800)
    lsb = (x >> np.uint32(12)) & np.uint32(1)
    r = (x + half - np.uint32(1) + lsb) & keep
    return r.view(np.float32)


def _split_f32r(w):
    hi = _f32r(np.ascontiguousarray(w, np.float32))
    lo = _f32r(w - hi)
    return hi, lo


def _split_bf16(w):
    import ml_dtypes
    hi = np.ascontiguousarray(np.asarray(w, np.float32).astype(ml_dtypes.bfloat16))
    lo = np.ascontiguousarray((w - hi.astype(np.float32)).astype(ml_dtypes.bfloat16))
    return hi, lo


def build_inputs(inp):
    """Host-side prep: per-core input maps (weights replicated)."""
    x = inp["x"]

    def thr_pack(g, b, bias):
        A = (2.0 - b) / g
        return np.ascontiguousarray(np.stack([A, bias], axis=1).astype(np.float32))

    wqkvT = np.ascontiguousarray(
        np.concatenate([inp["q_w"].T, inp["k_w"].T, inp["v_w"].T], axis=1))
    wqkv_hi, wqkv_lo = _split_f32r(wqkvT)
    wp_hi, wp_lo = _split_bf16(np.ascontiguousarray(inp["proj_w"].T))
    w1 = _f32r(np.ascontiguousarray(inp["fc1_w"].T))
    w2 = _f32r(np.ascontiguousarray(inp["fc2_w"].T))

    zc = np.zeros(C, np.float32)
    thr_qkv = np.ascontiguousarray(np.concatenate([
        thr_pack(inp["q_g"], inp["q_b"], zc),
        thr_pack(inp["k_g"], inp["k_b"], zc),
        thr_pack(inp["v_g"], inp["v_b"], zc)], axis=1))

    shared = dict(
        wqkvT_hi=wqkv_hi, wqkvT_lo=wqkv_lo,
        wprojT_hi=wp_hi, wprojT_lo=wp_lo,
        wfc1T=w1, wfc2T=w2, thr_qkv=thr_qkv,
        thr_proj=thr_pack(inp["proj_g"], inp["proj_b"], inp["proj_bias"]),
        thr_fc1=thr_pack(inp["fc1_g"], inp["fc1_b"], inp["fc1_bias"]),
        thr_fc2=thr_pack(inp["fc2_g"], inp["fc2_b"], inp["fc2_bias"]))

    in_maps = []
    for i in range(NCORES):
        xl_full = np.ascontiguousarray(
            np.concatenate([x[BPC * i + b] for b in range(BPC)], axis=1))
        x_hi, x_lo = _split_f32r(xl_full)
        in_maps.append(dict(x_hi=x_hi, x_lo=x_lo, x_f32=xl_full, **shared))
    return in_maps


def get_program():
    if "nc" not in _cache:
        _cache["nc"] = _build_program()
    return _cache["nc"]


def run(in_maps, **kwargs):
    _ensure_axon_hooks_shim()
    from concourse.bass_utils import run_bass_kernel_spmd
    nc = get_program()
    return run_bass_kernel_spmd(nc, in_maps, list(range(NCORES)), **kwargs)


def kernel(**inputs):
    inp = {k: np.asarray(v, dtype=np.float32) for k, v in inputs.items()}
    assert inp["x"].shape == (B, C, N), inp["x"].shape
    res = run(build_inputs(inp))
    out = np.empty((B, C, N), np.float32)
    for i in range(NCORES):
        o = res.results[i]["out"]
        for b in range(BPC):
            out[BPC * i + b] = o[:, N * b:N * (b + 1)]
    return out



# revision 8
# speedup vs baseline: 1.3100x; 1.3100x over previous
"""Trainium2 Bass kernel for nn_Block_17738214932786 (spiking transformer block).

Computation (B=16, C=512, N=1024, H=8 heads, HID=2048):
    q = spike(bn(q_w @ x)); k,v likewise          (spikes are 0/1)
    attn = (Qh Kh^T) Vh * 0.25 == Qh (Kh^T Vh) * 0.25   (exact: integers)
    a = spike(attn)                               (threshold attn >= 8)
    a = spike(bn(proj_w @ a + proj_bias))
    x = x + a
    h = spike(bn(fc1_w @ x + fc1_bias))
    h = spike(bn(fc2_w @ h + fc2_bias))
    out = x + h

Strategy: data-parallel over batch across 8 NeuronCores (2 batches/core,
per-core activation matrix [512, 2048]). BatchNorm (training mode: stats
over batch*length) is handled sync-BN style: per-channel [mean, E[y^2]]
AllGathered per conv (tiny buffers); BN+LIF then collapses to a
per-channel threshold compare y >= t.

Precision: qkv convs run as one f32r pass (hi) plus two fp8 DoubleRow
passes (w_lo*x_hi and w_hi*x_lo, operands pre-scaled into fp8 range,
accumulated in a second PSUM bank and combined as y = hi + 2^-16*lo).
This matches 3-pass f32r bit-for-bit on the spike outputs (verified in
emulation + HW probe) at 2/3 the PE cost. proj/fc1/fc2: single-pass
f32r. Attention exact (spikes 0/1; KtV integer counts split hi/lo bf16
losslessly). h1 never leaves SBUF: fc1 slices (4 m-tiles each) are
spiked in place to f32r and immediately consumed by interleaved fc2
partial sweeps accumulating into an SBUF y2 buffer. Emulated end-to-end
rel err 1.23e-2 (gate 2e-2).
"""

import sys
import types
import numpy as np

B, C, N, H = 16, 512, 1024, 8
D = C // H
HID = 4 * C
NCORES = 8
BPC = B // NCORES          # batches per core
COLS = BPC * N             # 2048
P = 128
NKC = C // P               # 4  tiles over C
NMH = HID // P             # 16 tiles over HID
NCH = COLS // 512          # 4  512-col chunks per core
NPT = COLS // P            # 16 col tiles per core
BN_EPS = 1e-5
LO_SCALE = float(2.0 ** -16)

_cache = {}


def _ensure_axon_hooks_shim():
    try:
        import antenv.axon_hooks  # noqa: F401
        return
    except Exception:
        pass
    m = types.ModuleType("antenv.axon_hooks")
    m.get_axon_ntff_profile_hook = lambda: None
    try:
        import antenv  # noqa: F401
    except Exception:
        sys.modules["antenv"] = types.ModuleType("antenv")
    sys.modules["antenv.axon_hooks"] = m


def _build_program():
    from contextlib import ExitStack
    import concourse.bacc as bacc
    import concourse.tile as tile
    from concourse import mybir
    from concourse.masks import make_identity

    dt = mybir.dt
    f32, bf16, f32r = dt.float32, dt.bfloat16, dt.float32r
    f8 = dt.float8e4
    AF = mybir.ActivationFunctionType
    GE = mybir.AluOpType.is_ge
    MUL = mybir.AluOpType.mult
    ADD = mybir.AluOpType.add
    DR = mybir.MatmulPerfMode.DoubleRow
    RG = [list(range(NCORES))]

    nc = bacc.Bacc("TRN2", target_bir_lowering=False, debug=False,
                   num_devices=NCORES)

    xr_in = nc.dram_tensor("x_r", [C, COLS], f32r, kind="ExternalInput")
    x8h_in = nc.dram_tensor("x8h", [C, COLS], f8, kind="ExternalInput")
    x8l_in = nc.dram_tensor("x8l", [C, COLS], f8, kind="ExternalInput")
    wqh_in = nc.dram_tensor("wqkvT_hi", [C, 3 * C], f32r, kind="ExternalInput")
    wq8l_in = nc.dram_tensor("wqkv8l", [C, 3 * C], f8, kind="ExternalInput")
    wq8h_in = nc.dram_tensor("wqkv8h", [C, 3 * C], f8, kind="ExternalInput")
    wp_in = nc.dram_tensor("wprojT", [C, C], f32r, kind="ExternalInput")
    wfc1_in = nc.dram_tensor("wfc1T", [C, HID], f32r, kind="ExternalInput")
    wfc2_in = nc.dram_tensor("wfc2T", [HID, C], f32r, kind="ExternalInput")
    thr_qkv_in = nc.dram_tensor("thr_qkv", [C, 6], f32, kind="ExternalInput")
    thr_proj_in = nc.dram_tensor("thr_proj", [C, 2], f32, kind="ExternalInput")
    thr_fc1_in = nc.dram_tensor("thr_fc1", [HID, 2], f32, kind="ExternalInput")
    thr_fc2_in = nc.dram_tensor("thr_fc2", [C, 2], f32, kind="ExternalInput")
    out_ext = nc.dram_tensor("out", [C, COLS], f32, kind="ExternalOutput")

    def part3(ap, p=P):  # [(m p), n] dram view -> [p, m, n]
        return ap.rearrange("(m p) n -> p m n", p=p)

    with tile.TileContext(nc, pool_alloc_mode="queue") as tc, ExitStack() as es:
        misc = es.enter_context(tc.tile_pool(name="misc", bufs=1))
        dram = es.enter_context(tc.tile_pool(name="dram", bufs=1, space="DRAM"))
        pp_mm = es.enter_context(tc.tile_pool(name="pp_mm", bufs=6, space="PSUM"))
        pp_sm = es.enter_context(tc.tile_pool(name="pp_sm", bufs=2, space="PSUM"))

        ident_bf = misc.tile([P, P], bf16)
        make_identity(nc, ident_bf)
        eps_t = misc.tile([P, 1], f32)
        nc.vector.memset(eps_t, BN_EPS)

        par_qkv = misc.tile([P, NKC, 6], f32)
        nc.gpsimd.dma_start(out=par_qkv, in_=part3(thr_qkv_in[:, :]))
        par_proj = misc.tile([P, NKC, 2], f32)
        nc.gpsimd.dma_start(out=par_proj, in_=part3(thr_proj_in[:, :]))
        par_fc1 = misc.tile([P, NMH, 2], f32)
        nc.gpsimd.dma_start(out=par_fc1, in_=part3(thr_fc1_in[:, :]))
        par_fc2 = misc.tile([P, NKC, 2], f32)
        nc.gpsimd.dma_start(out=par_fc2, in_=part3(thr_fc2_in[:, :]))

        # warmup collectives: absorb first-call staging/skew so the first
        # real stats AllGather runs at steady-state latency
        for wu in range(2):
            dmy_in = dram.tile([P, 2], f32, name=f"dmy_in{wu}")
            dmy_out = dram.tile([NCORES, P, 2], f32, name=f"dmy_out{wu}")
            nc.gpsimd.collective_compute(
                "AllGather", mybir.AluOpType.bypass, replica_groups=RG,
                ins=[dmy_in.opt()], outs=[dmy_out.opt()])

        def stats_finish(name, pool, stats, nm):
            mv = pool.tile([P, nm, 2], f32, name=f"mv_{name}")
            for m in range(nm):
                nc.vector.bn_aggr(out=mv[:, m, :], in_=stats[:, m, :, :])
            pack = pool.tile([P, nm, 2], f32, name=f"pk_{name}")
            nc.vector.tensor_mul(pack[:, :, 1], mv[:, :, 0], mv[:, :, 0])
            nc.vector.tensor_add(pack[:, :, 1], pack[:, :, 1], mv[:, :, 1])
            nc.vector.tensor_copy(pack[:, :, 0], mv[:, :, 0])
            bin_ = dram.tile([P, nm * 2], f32, name=f"arin_{name}")
            bout = dram.tile([NCORES, P, nm * 2], f32, name=f"arout_{name}")
            nc.sync.dma_start(out=bin_, in_=pack)
            nc.gpsimd.collective_compute(
                "AllGather", mybir.AluOpType.bypass, replica_groups=RG,
                ins=[bin_.opt()], outs=[bout.opt()])
            return bout

        def thresholds(name, pool, bout, thr_par, thr_col, nm):
            """AllGathered per-core stats -> reduce -> thresholds [P, nm]."""
            ag = pool.tile([P, NCORES, nm, 2], f32, name=f"ag_{name}")
            nc.sync.dma_start(out=ag, in_=bout.rearrange("r p c -> p r c"))
            nc.vector.tensor_add(ag[:, 0:4], ag[:, 0:4], ag[:, 4:8])
            nc.vector.tensor_add(ag[:, 0:2], ag[:, 0:2], ag[:, 2:4])
            arst = pool.tile([P, nm, 2], f32, name=f"ar_{name}")
            nc.vector.tensor_add(arst, ag[:, 0, :, :], ag[:, 1, :, :])
            nc.vector.tensor_scalar_mul(arst, arst, 1.0 / NCORES)
            t_t = pool.tile([P, nm], f32, name=f"thr_{name}")
            tmp = pool.tile([P, nm], f32, name=f"tmp_{name}")
            nc.vector.tensor_mul(tmp, arst[:, :, 0], arst[:, :, 0])
            nc.vector.tensor_sub(tmp, arst[:, :, 1], tmp)
            nc.scalar.activation(out=tmp, in_=tmp, func=AF.Sqrt,
                                 bias=eps_t, scale=1.0)
            nc.vector.tensor_mul(tmp, tmp, thr_par[:, :, thr_col])
            nc.vector.tensor_add(t_t, tmp, arst[:, :, 0])
            nc.vector.tensor_sub(t_t, t_t, thr_par[:, :, thr_col + 1])
            return t_t

        def spike(dst, src, t_t, tcol, eng=None):
            (eng or nc.vector).tensor_scalar(
                out=dst, in0=src, scalar1=t_t[:, tcol:tcol + 1],
                scalar2=None, op0=GE)

        def transposes(spkb, dstT):
            # bf16 PE transpose; the PSUM->SBUF copy casts to fp8 (0/1 exact)
            for m in range(NKC):
                for p_ in range(NPT):
                    pst = pp_sm.tile([P, P], bf16, name="ps_sm")
                    nc.tensor.transpose(pst, in_=spkb[:, m, P * p_:P * p_ + P],
                                        identity=ident_bf)
                    nc.any.tensor_copy(dstT[:, p_, P * m:P * m + P], pst)

        def phase_ab(a_spk, p_ab):
            q_spk = p_ab.tile([P, NKC, COLS], bf16)
            kT = p_ab.tile([P, NPT, C], f8)
            vT = p_ab.tile([P, NPT, C], f8)

            with tc.tile_pool(name="p_x", bufs=2) as p_x, \
                 tc.tile_pool(name="p_x8", bufs=2) as p_x8, \
                 tc.tile_pool(name="p_ykv", bufs=2) as p_ykv:
                with tc.tile_pool(name="p_w", bufs=2) as p_w:
                    def qkv_conv(ci, y_sb):
                        c0 = 512 * ci
                        wh = p_w.tile([P, NKC, C], f32r, name="w_hi", bufs=2)
                        for hf2 in range(2):
                            cc = c0 + 256 * hf2
                            (nc.sync, nc.scalar)[hf2].dma_start(
                                out=wh[:, :, 256 * hf2:256 * hf2 + 256],
                                in_=part3(wqh_in[:, :])[:, :, cc:cc + 256])
                        w8l = p_w.tile([P, NKC, C], f8, name="w_8l", bufs=2)
                        nc.scalar.dma_start(
                            out=w8l, in_=part3(wq8l_in[:, :])[:, :, c0:c0 + 512])
                        w8h = p_w.tile([P, NKC, C], f8, name="w_8h", bufs=2)
                        nc.sync.dma_start(
                            out=w8h, in_=part3(wq8h_in[:, :])[:, :, c0:c0 + 512])
                        st = misc.tile([P, NKC, NCH, 6], f32, name=f"st_qkv{ci}")
                        for hf in range(NCH):
                            cs = slice(512 * hf, 512 * hf + 512)
                            xr = p_x.tile([P, NKC, 512], f32r, name="xc",
                                          bufs=2)
                            nc.sync.dma_start(
                                out=xr, in_=part3(xr_in[:, :])[:, :, cs])
                            x8 = p_x8.tile([P, 2, NKC, 512], f8, name="x8c",
                                           bufs=2)
                            nc.scalar.dma_start(
                                out=x8[:, 0], in_=part3(x8h_in[:, :])[:, :, cs])
                            nc.scalar.dma_start(
                                out=x8[:, 1], in_=part3(x8l_in[:, :])[:, :, cs])
                            for m in range(NKC):
                                ms = slice(P * m, P * m + P)
                                ph = pp_mm.tile([P, 512], f32, name="ps_mm")
                                for k in range(NKC):
                                    nc.tensor.matmul(
                                        ph, lhsT=wh[:, k, ms], rhs=xr[:, k, :],
                                        start=(k == 0), stop=(k == NKC - 1))
                                pl = pp_mm.tile([P, 512], f32, name="ps_mm")
                                for j in range(2):
                                    nc.tensor.matmul(
                                        pl, lhsT=w8l[:, 2 * j:2 * j + 2, ms],
                                        rhs=x8[:, 0, 2 * j:2 * j + 2, :],
                                        start=(j == 0), stop=False,
                                        perf_mode=DR)
                                for j in range(2):
                                    nc.tensor.matmul(
                                        pl, lhsT=w8h[:, 2 * j:2 * j + 2, ms],
                                        rhs=x8[:, 1, 2 * j:2 * j + 2, :],
                                        start=False, stop=(j == 1),
                                        perf_mode=DR)
                                # y = hi + 2^-16*lo; only one PSUM operand
                                # allowed per ALU op: ACT scales lo into
                                # SBUF, then add the hi bank in place
                                nc.scalar.activation(
                                    out=y_sb[:, m, cs], in_=pl,
                                    func=AF.Copy, scale=LO_SCALE)
                                nc.vector.tensor_add(y_sb[:, m, cs],
                                                     y_sb[:, m, cs], ph)
                                nc.vector.bn_stats(out=st[:, m, hf, :],
                                                   in_=y_sb[:, m, cs])
                        return stats_finish(f"qkv{ci}", misc, st, NKC)

                    y_k = p_ykv.tile([P, NKC, COLS], f32, name="ybuf")
                    bout_k = qkv_conv(1, y_k)
                    y_v = p_ykv.tile([P, NKC, COLS], f32, name="ybuf")
                    bout_v = qkv_conv(2, y_v)
                    # k spikes (bf16) + transposes (AG-k already done)
                    t_k = thresholds("k", misc, bout_k, par_qkv, 2, NKC)
                    for m in range(NKC):
                        spike(q_spk[:, m, :], y_k[:, m, :], t_k, m)
                    transposes(q_spk, kT)
                    y_q = p_ykv.tile([P, NKC, COLS], f32, name="ybuf")
                    bout_q = qkv_conv(0, y_q)
                    # v spikes + transposes (AG-v hidden under q conv)
                    t_v = thresholds("v", misc, bout_v, par_qkv, 4, NKC)
                    for m in range(NKC):
                        spike(q_spk[:, m, :], y_v[:, m, :], t_v, m)
                    transposes(q_spk, vT)
                    # q spikes (DVE waits AG-q; PE continues with
                    # v-transposes and the kv matmuls of phase B)
                    t_q = thresholds("q", misc, bout_q, par_qkv, 0, NKC)
                    for m in range(NKC):
                        spike(q_spk[:, m, :], y_q[:, m, :], t_q, m)

            # ---- Phase B: attention (exact integer bf16/fp8) ----
            with tc.tile_pool(name="p_kv", bufs=4) as p_kv:
                kvs = {}
                for b in range(BPC):
                    for j in range(H // 2):   # head pairs -> blockdiag lhsT
                        blk_hi = p_kv.tile([P, P], bf16, name="kvblk_hi")
                        blk_lo = p_kv.tile([P, P], bf16, name="kvblk_lo")
                        nc.gpsimd.memset(blk_hi, 0.0)
                        nc.gpsimd.memset(blk_lo, 0.0)
                        pkv = pp_sm.tile([P, 64], f32, name="ps_sm")
                        for hh in range(2):
                            h_ = 2 * j + hh
                            sl = slice(64 * hh, 64 * hh + 64)
                            for t_ in range(N // P):
                                nc.tensor.matmul(
                                    pkv[sl, :],
                                    lhsT=kT[:, (N // P) * b + t_, D * h_:D * h_ + D],
                                    rhs=vT[:, (N // P) * b + t_, D * h_:D * h_ + D],
                                    start=(t_ == 0), stop=(t_ == N // P - 1),
                                    tile_position=(0, 64 * hh))
                            # lossless integer split: hi=bf16(kv), lo=kv-hi
                            nc.any.tensor_copy(blk_hi[sl, sl], pkv[sl, :])
                            nc.vector.tensor_sub(blk_lo[sl, sl], pkv[sl, :],
                                                 blk_hi[sl, sl])
                        kvs[(b, j)] = (blk_hi, blk_lo)

                for b in range(BPC):
                    for j in range(H // 2):
                        blk_hi, blk_lo = kvs[(b, j)]
                        pas = [pp_mm.tile([P, 512], f32, name="ps_mm")
                               for _ in range(N // 512)]
                        for wi, blk in enumerate((blk_hi, blk_lo)):
                            for n_ in range(N // 512):
                                cs = slice(N * b + 512 * n_, N * b + 512 * n_ + 512)
                                nc.tensor.matmul(pas[n_], lhsT=blk,
                                                 rhs=q_spk[:, j, cs],
                                                 start=(wi == 0), stop=(wi == 1))
                        for n_ in range(N // 512):
                            cs = slice(N * b + 512 * n_, N * b + 512 * n_ + 512)
                            nc.vector.tensor_scalar(
                                out=a_spk[:, j, cs], in0=pas[n_],
                                scalar1=8.0, scalar2=None, op0=GE)

        def phase_c(a_spk, p_pr):
            # ---- Phase C: proj (1-pass f32r) + fused spike+residual.
            # xrr overwrites the a_spk tile (WAR dep after proj matmuls). ----
            if True:
                wpT = p_pr.tile([P, NKC, C], f32r)
                nc.sync.dma_start(out=wpT, in_=part3(wp_in[:, :]))
                xr_res = p_pr.tile([P, NKC, COLS], f32r, name="xr_res")
                for hf in range(NCH):
                    cs = slice(512 * hf, 512 * hf + 512)
                    nc.gpsimd.dma_start(out=xr_res[:, :, cs],
                                        in_=part3(xr_in[:, :])[:, :, cs])
                xrr = a_spk

                y_p = p_pr.tile([P, NKC, COLS], f32)
                st_p = misc.tile([P, NKC, NCH, 6], f32, name="st_proj")
                for m in range(NKC):
                    ms = slice(P * m, P * m + P)
                    pss = [pp_mm.tile([P, 512], f32, name="ps_mm")
                           for _ in range(NCH)]
                    for k in range(NKC):
                        for n_ in range(NCH):
                            nc.tensor.matmul(
                                pss[n_], lhsT=wpT[:, k, ms],
                                rhs=a_spk[:, k, 512 * n_:512 * n_ + 512],
                                start=(k == 0), stop=(k == NKC - 1))
                    for n_ in range(NCH):
                        cs = slice(512 * n_, 512 * n_ + 512)
                        nc.any.tensor_copy(y_p[:, m, cs], pss[n_])
                        nc.vector.bn_stats(out=st_p[:, m, n_, :], in_=pss[n_])
                bout_p = stats_finish("proj", misc, st_p, NKC)
                # PE<->DVE ping-pong keep-alive through the AllGather wait
                wka = p_pr.tile([P, P], bf16, name="wka")
                nc.vector.tensor_copy(wka, ident_bf)
                wpsk = pp_sm.tile([P, P], f32, name="ps_sm")
                for _ in range(10):
                    nc.tensor.matmul(wpsk, lhsT=wka, rhs=wka,
                                     start=True, stop=True)
                    nc.vector.tensor_copy(wka, wpsk)
                t_p = thresholds("proj", misc, bout_p, par_proj, 0, NKC)
                # xrr = (y_p >= t) + x_r, rounded to f32r (fused)
                for m in range(NKC):
                    nc.vector.scalar_tensor_tensor(
                        out=xrr[:, m, :], in0=y_p[:, m, :],
                        scalar=t_p[:, m:m + 1], in1=xr_res[:, m, :],
                        op0=GE, op1=ADD)
            return xrr

        def phase_de(xrr):
            # ====== fc1 slices interleaved with fc2 partial sweeps ======
            # fc1 slice s (4 m-tiles) -> stats AG_s -> spike in place to
            # f32r -> fc2 sweep s accumulates W2[:, slice]*h1[slice] into
            # y2 (SBUF). h1 never exists beyond two live slices.
            NSL = 4
            with tc.tile_pool(name="p_f2", bufs=1) as p_f2, \
                 tc.tile_pool(name="p_w1", bufs=2) as p_w1, \
                 tc.tile_pool(name="p_f1q", bufs=2) as p_f1q:
                wfc2T = p_f2.tile([P, NMH, C], f32r)
                for sl_ in range(4):
                    nc.gpsimd.dma_start(
                        out=wfc2T[:, 4 * sl_:4 * sl_ + 4, :],
                        in_=part3(wfc2_in[:, :])[:, 4 * sl_:4 * sl_ + 4, :])
                y2 = p_f2.tile([P, NKC, COLS], f32)
                st2 = misc.tile([P, NKC, NCH, 6], f32, name="st_fc2")

                # PE warm-keeper across the proj-AG bridge
                warm = misc.tile([P, 512], bf16, name="warm")
                nc.vector.tensor_copy(warm[:, 0:P], ident_bf)
                wps = pp_sm.tile([P, P], f32, name="ps_sm")
                for _ in range(16):
                    nc.tensor.matmul(wps, lhsT=warm[:, 0:P],
                                     rhs=warm[:, 0:P], start=True, stop=True)

                w1s = []
                for s in range(2):  # prefetch first two weight slices
                    w1 = p_w1.tile([P, NKC, 512], f32r, name="w1q", bufs=2)
                    (nc.sync, nc.scalar)[s].dma_start(
                        out=w1, in_=part3(wfc1_in[:, :])[:, :, 512 * s:512 * s + 512])
                    w1s.append(w1)

                def fc1_slice(s):
                    if s + 2 < NSL:
                        w1n = p_w1.tile([P, NKC, 512], f32r, name="w1q",
                                        bufs=2)
                        (nc.sync, nc.scalar)[s % 2].dma_start(
                            out=w1n,
                            in_=part3(wfc1_in[:, :])[:, :, 512 * (s + 2):512 * (s + 2) + 512])
                        w1s.append(w1n)
                    w1 = w1s[s]
                    y1q = p_f1q.tile([P, NKC, COLS], f32r, name="y1q", bufs=2)
                    st_q = misc.tile([P, NKC, NCH, 6], f32, name=f"st_fc1q{s}")
                    for mi in range(NKC):
                        pss = [pp_mm.tile([P, 512], f32, name="ps_mm")
                               for _ in range(NCH)]
                        for k in range(NKC):
                            for n_ in range(NCH):
                                nc.tensor.matmul(
                                    pss[n_],
                                    lhsT=w1[:, k, P * mi:P * mi + P],
                                    rhs=xrr[:, k, 512 * n_:512 * n_ + 512],
                                    start=(k == 0), stop=(k == NKC - 1))
                        for n_ in range(NCH):
                            cs = slice(512 * n_, 512 * n_ + 512)
                            nc.any.tensor_copy(y1q[:, mi, cs], pss[n_])
                            nc.vector.bn_stats(out=st_q[:, mi, n_, :],
                                               in_=pss[n_])
                    bout = stats_finish(f"fc1q{s}", misc, st_q, NKC)
                    return y1q, bout

                def fc1_finish(s, y1q, bout):
                    t1q = thresholds(f"fc1q{s}", misc, bout,
                                     par_fc1[:, 4 * s:4 * s + 4, :], 0, NKC)
                    for mi in range(NKC):
                        spike(y1q[:, mi, :], y1q[:, mi, :], t1q, mi)
                    return y1q

                def fc2_sweep(s, h1q):
                    for m in range(NKC):
                        ms = slice(P * m, P * m + P)
                        for n_ in range(NCH):
                            cs = slice(512 * n_, 512 * n_ + 512)
                            ps = pp_mm.tile([P, 512], f32, name="ps_mm")
                            for k in range(NKC):
                                nc.tensor.matmul(
                                    ps, lhsT=wfc2T[:, 4 * s + k, ms],
                                    rhs=h1q[:, k, cs],
                                    start=(k == 0), stop=(k == NKC - 1))
                            if s == 0:
                                nc.any.tensor_copy(y2[:, m, cs], ps)
                            else:
                                if (m + n_) % 2 == 0:
                                    nc.vector.tensor_add(y2[:, m, cs],
                                                         y2[:, m, cs], ps)
                                else:
                                    tmp = p_w1.tile([P, 512], f32, name="f2t",
                                                    bufs=2)
                                    nc.scalar.activation(out=tmp, in_=ps,
                                                         func=AF.Copy,
                                                         scale=1.0)
                                    nc.gpsimd.tensor_add(y2[:, m, cs],
                                                         y2[:, m, cs], tmp)
                                if s == NSL - 1:
                                    nc.vector.bn_stats(out=st2[:, m, n_, :],
                                                       in_=y2[:, m, cs])

                pend = None
                for s in range(NSL):
                    y1q, bout = fc1_slice(s)
                    if pend is not None:
                        ps_, py1q, pbout = pend
                        h1q = fc1_finish(ps_, py1q, pbout)
                        fc2_sweep(ps_, h1q)
                    pend = (s, y1q, bout)
                ps_, py1q, pbout = pend
                h1q = fc1_finish(ps_, py1q, pbout)
                fc2_sweep(ps_, h1q)

                # ====== tail: fc2 stats -> AG -> fused spike+residual ======
                bout2 = stats_finish("fc2", misc, st2, NKC)
                t2 = thresholds("fc2", misc, bout2, par_fc2, 0, NKC)
                out3 = part3(out_ext[:, :])
                for n_ in range(NCH):
                    cs = slice(512 * n_, 512 * n_ + 512)
                    for m in range(NKC):
                        nc.vector.scalar_tensor_tensor(
                            out=y2[:, m, cs], in0=y2[:, m, cs],
                            scalar=t2[:, m:m + 1], in1=xrr[:, m, cs],
                            op0=GE, op1=ADD)
                        (nc.sync, nc.scalar)[(m + n_) % 2].dma_start(
                            out=out3[:, m, cs], in_=y2[:, m, cs])

        with tc.tile_pool(name="p_as", bufs=1) as p_as:  # a_spk/xrr: A..E
            a_spk = p_as.tile([P, NKC, COLS], f32r)
            with tc.tile_pool(name="p_ab", bufs=1) as p_ab:  # lives A..B
                phase_ab(a_spk, p_ab)
            with tc.tile_pool(name="p_pr", bufs=1) as p_pr:  # lives C
                xrr = phase_c(a_spk, p_pr)
            phase_de(xrr)

    nc.compile()
    return nc


def _f32r(v):
    """Round float32 array to f32r (11-bit mantissa, RNE) - bit-exact vs
    the TRN2 DVE cast (verified on hardware)."""
    x = np.ascontiguousarray(v, np.float32).view(np.uint32)
    keep = np.uint32(0xFFFFF000)
    half = np.uint32(0x800)
    lsb = (x >> np.uint32(12)) & np.uint32(1)
    r = (x + half - np.uint32(1) + lsb) & keep
    return r.view(np.float32)


def build_inputs(inp):
    """Host-side prep: per-core input maps (weights replicated)."""
    import ml_dtypes
    f8 = ml_dtypes.float8_e4m3
    x = inp["x"]

    def thr_pack(g, b, bias):
        A = (2.0 - b) / g
        return np.ascontiguousarray(np.stack([A, bias], axis=1).astype(np.float32))

    wqkvT = np.ascontiguousarray(
        np.concatenate([inp["q_w"].T, inp["k_w"].T, inp["v_w"].T], axis=1))
    wq_hi = _f32r(wqkvT)
    wq_lo = wqkvT - wq_hi
    wq8l = np.ascontiguousarray((wq_lo * 65536.0).astype(f8))
    wq8h = np.ascontiguousarray((wq_hi * 16.0).astype(f8))
    wp = _f32r(np.ascontiguousarray(inp["proj_w"].T))
    w1 = _f32r(np.ascontiguousarray(inp["fc1_w"].T))
    w2 = _f32r(np.ascontiguousarray(inp["fc2_w"].T))

    zc = np.zeros(C, np.float32)
    thr_qkv = np.ascontiguousarray(np.concatenate([
        thr_pack(inp["q_g"], inp["q_b"], zc),
        thr_pack(inp["k_g"], inp["k_b"], zc),
        thr_pack(inp["v_g"], inp["v_b"], zc)], axis=1))

    shared = dict(
        wqkvT_hi=wq_hi, wqkv8l=wq8l, wqkv8h=wq8h,
        wprojT=wp, wfc1T=w1, wfc2T=w2, thr_qkv=thr_qkv,
        thr_proj=thr_pack(inp["proj_g"], inp["proj_b"], inp["proj_bias"]),
        thr_fc1=thr_pack(inp["fc1_g"], inp["fc1_b"], inp["fc1_bias"]),
        thr_fc2=thr_pack(inp["fc2_g"], inp["fc2_b"], inp["fc2_bias"]))

    in_maps = []
    for i in range(NCORES):
        xl_full = np.ascontiguousarray(
            np.concatenate([x[BPC * i + b] for b in range(BPC)], axis=1))
        x_r = _f32r(xl_full)
        x_lo = xl_full - x_r
        in_maps.append(dict(
            x_r=x_r,
            x8h=np.ascontiguousarray(x_r.astype(f8)),
            x8l=np.ascontiguousarray((x_lo * 4096.0).astype(f8)),
            **shared))
    return in_maps


def get_program():
    if "nc" not in _cache:
        _cache["nc"] = _build_program()
    return _cache["nc"]


def run(in_maps, **kwargs):
    _ensure_axon_hooks_shim()
    from concourse.bass_utils import run_bass_kernel_spmd
    nc = get_program()
    return run_bass_kernel_spmd(nc, in_maps, list(range(NCORES)), **kwargs)


def kernel(**inputs):
    inp = {k: np.asarray(v, dtype=np.float32) for k, v in inputs.items()}
    assert inp["x"].shape == (B, C, N), inp["x"].shape
    res = run(build_inputs(inp))
    out = np.empty((B, C, N), np.float32)
    for i in range(NCORES):
        o = res.results[i]["out"]
        for b in range(BPC):
            out[BPC * i + b] = o[:, N * b:N * (b + 1)]
    return out
